# revision 1
# baseline (speedup 1.0000x reference)
"""Trainium2 Bass kernel for the supervoxel erode/edge loss module.

The reference divides a padded [B,X,Y] grid (pad offset 4*sx along x, 4*sy
along y) into 8x8 patches, zeroes the last row/col of the mask channel in
each patch, erodes along both patch axes and sums eroded*edge. The erode
`a*b + (1-a)*a + (1-b)*a` algebraically equals `2a - a^2` with
a = m(i)*m(i+1) (the second operand cancels), and because both the patch
shifts and the patch-boundary zeroing are local, the whole module collapses
to a global elementwise expression on the unpadded grid:

    mt(x,y) = mask[b,x,y,idx] * [(x+4sx)%8 != 7] * [(y+4sy)%8 != 7]
    ax = mt(x,y)*mt(x+1,y); ay = mt(x,y)*mt(x,y+1)   (zero past image edge)
    total = sum_b,x,y ax(2-ax) * ay(2-ay) * edge
    out = loss_old + total / (B * ((X+8)//8) * ((Y+8)//8))

With raw products ax0 = raw(x)raw(x+1), ay0 = raw(x,y)raw(x,y+1) the masks
fold out of the elementwise chain:

    contribution = ax0(2-ax0) * ay0(2-ay0) * edge * R(x) * C(y)

R(x) = [x%8 not in {6-4sx, 7-4sx}] is applied to the final per-row partial
sums, and C(y) = [y%8 not in {6-4sy, 7-4sy}] by restricting the elementwise
ops to the live columns of each 8-group (sy==0), or by one extra multiply.

x-tiles are 121 rows at stride 120 (one-row overlap so the x-neighbor
product never crosses a tile boundary; 120 % 8 == 0 keeps R per-partition
tile-invariant). DMA is the roofline: per-transfer fixed cost serializes on
the queue rings, so mask tiles are loaded two-at-a-time with one
overlapping-window DMA (~3.9 MiB each) and edge as one whole-image DMA.

Per x-tile the compute pipeline is:
    PE    : shifted = S @ v  (S = shift-by-one-row matrix; v = stride-4
            channel view of the mask tile)
    DVE   : ax0 = v*shifted, nx = (ax0-2)*ax0, ny = (ay0-2)*ay0, reduce
    Pool  : ay0 = v*v(y+1), p1 = nx*ny, p2 = p1*edge
    ((a-2)*a = -(a(2-a)); the two negations cancel in p1 = nx*ny.)

Sharding: data-parallel over batch, B/8 images per core on 8 cores; each
core returns a masked partial sum, combined on host (the mean is a single
scalar, so no device collective is needed).
"""

import sys

sys.path.insert(0, "/opt/trn_rl_repo")

import numpy as np

from concourse import bacc, bass, mybir, tile
from concourse.ap import AP
from concourse.bass_utils import run_bass_kernel_spmd

F32 = mybir.dt.float32
N_CORES = 8
TS = 120  # x-tile stride (multiple of 8 so the %8 row pattern is tile-invariant)
SHIFTS = [(0, 0), (1, 0), (0, 1), (1, 1)]


def _build_program(
    Bc: int,
    X: int,
    Y: int,
    idx: int,
    sy: int,
    niter: int = 1,
    variant: str = "full",
    dma_mode: str = "gpsimd",
):
    """Build the per-core Bass program. Inputs (per core):
    mask [Bc,X,Y,4] f32, edge [Bc,X,Y,1] f32, smat [128,128], rvec [128,1],
    cvec [128,Y] (used only when sy != 0). Output: out [1,1] f32 partial sum.
    niter > 1 repeats the whole computation on-device (timing only).
    """
    assert X % 8 == 0 and Y % 8 == 0
    nk = (X + TS - 1) // TS  # x-tiles per image
    nt = Bc * nk  # total tiles
    npair = nk // 2
    odd_rows = X - 2 * TS * npair  # rows of the trailing unpaired tile (0 if none)
    G = Y // 8
    packed = sy == 0  # live cols are j in 0..5 of every group of 8

    nc = bacc.Bacc("TRN2", target_bir_lowering=False, debug=False)
    mask_h = nc.dram_tensor("mask", [Bc, X, Y, 4], F32, kind="ExternalInput")
    edge_h = nc.dram_tensor("edge", [Bc, X, Y, 1], F32, kind="ExternalInput")
    smat_h = nc.dram_tensor("smat", [128, 128], F32, kind="ExternalInput")
    rvec_h = nc.dram_tensor("rvec", [128, 1], F32, kind="ExternalInput")
    cvec_h = nc.dram_tensor("cvec", [128, Y], F32, kind="ExternalInput")
    out_h = nc.dram_tensor("out", [1, 1], F32, kind="ExternalOutput")

    if dma_mode == "gpsimd":
        eng_mask, eng_edge = "gpsimd", "sync"
    elif dma_mode == "sync":
        eng_mask, eng_edge = "sync", "scalar"
    else:
        eng_mask, eng_edge = "scalar", "sync"

    def mask_pair_src(b, m):
        """Overlapping-window DRAM AP: [121, 2, Y, 4] where element
        (p, j, y, c) reads mask[b, 2*TS*m + TS*j + p, y, c]."""
        row = Y * 4  # elements per x-row
        off = (b * X + 2 * TS * m) * row
        ap = [[row, TS + 1], [TS * row, 2], [4, Y], [1, 4]]
        return AP(mask_h, off, ap)

    with tile.TileContext(nc) as tc:
        with (
            tc.tile_pool(name="mt", bufs=2) as mt_pool,
            tc.tile_pool(name="et", bufs=2) as et_pool,
            tc.tile_pool(name="work", bufs=2) as w_pool,
            tc.tile_pool(name="pp", bufs=2) as p_pool,
            tc.tile_pool(name="psum", bufs=2, space="PSUM") as ps_pool,
            tc.tile_pool(name="psum1", bufs=1, space="PSUM") as ps1_pool,
            tc.tile_pool(name="const", bufs=1) as c_pool,
        ):
            smat_t = c_pool.tile([128, 128], F32)
            rvec_t = c_pool.tile([128, 1], F32)
            ones_t = c_pool.tile([128, 1], F32)
            partials = c_pool.tile([128, nt], F32)
            nc.sync.dma_start(smat_t[:], smat_h.ap())
            nc.sync.dma_start(rvec_t[:], rvec_h.ap())
            nc.gpsimd.memset(ones_t[:], 1.0)
            cvec_t = None
            if not packed:
                cvec_t = c_pool.tile([128, Y], F32)
                nc.sync.dma_start(cvec_t[:], cvec_h.ap())

            def emit_compute(v, et_v, cr, t_idx):
                """v: [rows>=cr(+1), Y] stride-4 mask-channel view;
                et_v: [cr, Y] edge view; accumulates into partials[:, t_idx]."""
                rows = v.shape[0]
                if variant == "dma":
                    # timing ablation: loads only, tiny consumer so nothing is elided
                    nc.vector.reduce_sum(
                        partials[0:1, t_idx : t_idx + 1],
                        v[0:1, 0:8],
                        axis=mybir.AxisListType.X,
                    )
                    nc.gpsimd.tensor_mul(
                        partials[0:1, t_idx : t_idx + 1],
                        partials[0:1, t_idx : t_idx + 1],
                        et_v[0:1, 0:1],
                    )
                    return
                shifted = ps_pool.tile([128, Y], F32)
                if variant != "pool":
                    for c0 in range(0, Y, 512):
                        cw = min(512, Y - c0)
                        nc.tensor.matmul(
                            shifted[:, c0 : c0 + cw],
                            smat_t[0:rows, :],
                            v[:, c0 : c0 + cw],
                            start=True,
                            stop=True,
                        )

                if packed:
                    ax0 = w_pool.tile([cr, G, 6], F32)
                    ay0 = w_pool.tile([cr, G, 6], F32)
                    nxt = w_pool.tile([cr, G, 6], F32)
                    nyt = w_pool.tile([cr, G, 6], F32)
                    p1 = p_pool.tile([cr, G, 6], F32)
                    p2 = p_pool.tile([cr, G, 6], F32)

                    def lv(t, j0=0, j1=6):
                        return t.rearrange("p (g j) -> p g j", j=8)[:, :, j0:j1]

                    v_l = lv(v[0:cr, :])
                    v_l1 = lv(v[0:cr, :], 1, 7)  # col + 1
                    sh_l = lv(shifted[0:cr, :])
                    if variant == "dve":
                        nc.vector.tensor_mul(ax0[:], v_l, sh_l)
                        nc.vector.scalar_tensor_tensor(
                            nxt[:], ax0[:], 2.0, ax0[:],
                            op0=mybir.AluOpType.subtract, op1=mybir.AluOpType.mult,
                        )
                        nc.vector.scalar_tensor_tensor(
                            nyt[:], nxt[:], 2.0, nxt[:],
                            op0=mybir.AluOpType.subtract, op1=mybir.AluOpType.mult,
                        )
                        nc.vector.reduce_sum(
                            partials[0:cr, t_idx : t_idx + 1], nyt[:],
                            axis=mybir.AxisListType.XY,
                        )
                        return
                    if variant == "pool":
                        nc.gpsimd.tensor_mul(ay0[:], v_l, v_l1)
                        nc.gpsimd.tensor_mul(p1[:], ay0[:], ay0[:])
                        nc.gpsimd.tensor_mul(p2[:], p1[:], lv(et_v))
                        nc.vector.reduce_sum(
                            partials[0:cr, t_idx : t_idx + 1], p2[:],
                            axis=mybir.AxisListType.XY,
                        )
                        return
                    # ax0 = v * (v shifted one row); ay0 = v * (v shifted one col)
                    nc.vector.tensor_mul(ax0[:], v_l, sh_l)
                    nc.gpsimd.tensor_mul(ay0[:], v_l, v_l1)
                    # n = (a - 2) * a = -e; the negations cancel in the product
                    nc.vector.scalar_tensor_tensor(
                        nxt[:], ax0[:], 2.0, ax0[:],
                        op0=mybir.AluOpType.subtract, op1=mybir.AluOpType.mult,
                    )
                    nc.vector.scalar_tensor_tensor(
                        nyt[:], ay0[:], 2.0, ay0[:],
                        op0=mybir.AluOpType.subtract, op1=mybir.AluOpType.mult,
                    )
                    nc.gpsimd.tensor_mul(p1[:], nxt[:], nyt[:])
                    nc.gpsimd.tensor_mul(p2[:], p1[:], lv(et_v))
                    nc.vector.reduce_sum(
                        partials[0:cr, t_idx : t_idx + 1], p2[:],
                        axis=mybir.AxisListType.XY,
                    )
                else:
                    W = Y - 1
                    ax0 = w_pool.tile([cr, Y], F32)
                    ay0 = w_pool.tile([cr, Y], F32)
                    nxt = w_pool.tile([cr, Y], F32)
                    nyt = w_pool.tile([cr, Y], F32)
                    p1 = p_pool.tile([cr, Y], F32)
                    p2 = p_pool.tile([cr, Y], F32)
                    nc.vector.tensor_mul(ax0[:, 0:W], v[0:cr, 0:W], shifted[0:cr, 0:W])
                    nc.gpsimd.tensor_mul(ay0[:, 0:W], v[0:cr, 0:W], v[0:cr, 1:Y])
                    # fold the column mask into ay0 (C is 0/1 so e_y picks it up)
                    nc.gpsimd.tensor_mul(ay0[:, 0:W], ay0[:, 0:W], cvec_t[0:cr, 0:W])
                    nc.vector.scalar_tensor_tensor(
                        nxt[:, 0:W], ax0[:, 0:W], 2.0, ax0[:, 0:W],
                        op0=mybir.AluOpType.subtract, op1=mybir.AluOpType.mult,
                    )
                    nc.vector.scalar_tensor_tensor(
                        nyt[:, 0:W], ay0[:, 0:W], 2.0, ay0[:, 0:W],
                        op0=mybir.AluOpType.subtract, op1=mybir.AluOpType.mult,
                    )
                    nc.gpsimd.tensor_mul(p1[:, 0:W], nxt[:, 0:W], nyt[:, 0:W])
                    nc.gpsimd.tensor_mul(p2[:, 0:W], p1[:, 0:W], et_v[:, 0:W])
                    nc.vector.reduce_sum(
                        partials[0:cr, t_idx : t_idx + 1], p2[:, 0:W],
                        axis=mybir.AxisListType.X,
                    )

            def emit_iter():
                nc.vector.memset(partials[:], 0.0)
                for b in range(Bc):
                    # one DMA for all full x-tiles' edge rows, one for the tail
                    etm = et_pool.tile([TS, 2 * npair, Y], F32)
                    getattr(nc, eng_edge).dma_start(
                        etm[:],
                        edge_h.ap()[b, 0 : 2 * TS * npair, :, 0].rearrange(
                            "(k p) y -> p k y", p=TS
                        ),
                    )
                    eto = None
                    if odd_rows:
                        eto = et_pool.tile([odd_rows, Y], F32)
                        getattr(nc, eng_edge).dma_start(
                            eto[:], edge_h.ap()[b, 2 * TS * npair : X, :, 0]
                        )
                    for m in range(npair):
                        mtp = mt_pool.tile([TS + 1, 2, Y, 4], F32)
                        getattr(nc, eng_mask).dma_start(mtp[:], mask_pair_src(b, m))
                        for j in range(2):
                            k = 2 * m + j
                            emit_compute(
                                mtp[:, j, :, idx], etm[:, k, :], TS, b * nk + k
                            )
                    if odd_rows:
                        mto = mt_pool.tile([odd_rows, Y, 4], F32)
                        getattr(nc, eng_mask).dma_start(
                            mto[:], mask_h.ap()[b, 2 * TS * npair : X, :, :]
                        )
                        emit_compute(
                            mto[:, :, idx], eto[:], odd_rows, b * nk + nk - 1
                        )
                # total = sum_p rvec[p] * sum_t partials[p, t]
                red = c_pool.tile([128, 1], F32)
                rm = c_pool.tile([128, 1], F32)
                nc.vector.reduce_sum(red[:], partials[:], axis=mybir.AxisListType.X)
                nc.vector.tensor_mul(rm[:], red[:], rvec_t[:])
                out_ps = ps1_pool.tile([1, 1], F32)
                nc.tensor.matmul(out_ps[:], rm[:], ones_t[:], start=True, stop=True)
                out_sb = c_pool.tile([1, 1], F32)
                nc.vector.tensor_copy(out_sb[:], out_ps[:])
                nc.sync.dma_start(out_h.ap(), out_sb[:])

            if niter == 1:
                emit_iter()
            else:
                with tc.For_i(0, niter, 1):
                    emit_iter()

    nc.compile()
    return nc


def _host_consts(idx: int):
    sx, sy = SHIFTS[idx]
    smat = np.zeros((128, 128), np.float32)
    for p in range(127):
        smat[p + 1, p] = 1.0
    xs = np.arange(128)
    rvec = (
        (((xs + 4 * sx) % 8 != 7) & ((xs + 1 + 4 * sx) % 8 != 7))
        .astype(np.float32)
        .reshape(128, 1)
    )
    return smat, rvec


def _host_cvec(idx: int, Y: int):
    _, sy = SHIFTS[idx]
    ys = np.arange(Y)
    cv = (((ys + 4 * sy) % 8 != 7) & ((ys + 1 + 4 * sy) % 8 != 7)).astype(np.float32)
    return np.broadcast_to(cv, (128, Y)).copy()


def _run(mask, edge, loss_old, idx, trace=False, niter=1, **build_kwargs):
    B, X, Y, _ = mask.shape
    assert B % N_CORES == 0
    Bc = B // N_CORES
    sx, sy = SHIFTS[idx]

    nc = _build_program(Bc, X, Y, idx, sy, niter=niter, **build_kwargs)
    smat, rvec = _host_consts(idx)
    cvec = _host_cvec(idx, Y)
    in_maps = [
        {
            "mask": mask[i * Bc : (i + 1) * Bc],
            "edge": edge[i * Bc : (i + 1) * Bc],
            "smat": smat,
            "rvec": rvec,
            "cvec": cvec,
        }
        for i in range(N_CORES)
    ]
    res = run_bass_kernel_spmd(nc, in_maps, list(range(N_CORES)), trace=trace)
    total = float(sum(float(res.results[i]["out"][0, 0]) for i in range(N_CORES)))
    n_patch = ((X + 8) // 8) * ((Y + 8) // 8)
    out = np.float32(np.asarray(loss_old, dtype=np.float32) + total / (B * n_patch))
    return np.asarray(out, dtype=np.float32), res


def kernel(resized_image, mask_combined, edge_map, loss_old, mask_index):
    mask = np.ascontiguousarray(np.asarray(mask_combined, dtype=np.float32))
    edge = np.ascontiguousarray(np.asarray(edge_map, dtype=np.float32))
    idx = int(np.asarray(mask_index))
    out, _ = _run(mask, edge, loss_old, idx)
    return out

